# revision 47
# baseline (speedup 1.0000x reference)
"""F8Linear as a column-parallel hybrid fp8/bf16 GEMM across 8 NeuronCores.

y = x @ (w_f8 * w_scale).T + bias
  x: [2, 512, 4096] bf16, w_f8: [14336, 4096] f32 (fp8-representable values),
  w_scale: scalar f32, bias: [14336] f32 -> y: [2, 512, 14336] bf16

Sharding: column-parallel - each core owns 1792 out-features; x replicated;
host gathers the 8 output slices. No collectives.

Speed trick: the TRN2 PE runs fp8e4 matmuls in DoubleRow mode at 2x the bf16
k-throughput (measured 220ns per FD=512 matmul contracting 256 k, vs 215ns
per 128 k in bf16). The weights are exactly fp8-representable: w_f8/2 lands
in TRN fp8e4 range (|w|<=224<240), so the weight side is EXACT in fp8 and
one fp32 scale (2*w_scale) is applied at PSUM drain. Only x must be
quantized to e4m3 for the DoubleRow moving operand, which costs ~2.7% rms
error if applied to all of K - too much for the 2e-2 gate. So K is split:
the first F=8 k-tiles of 256 (2048 k) run as fp8 DoubleRow; the remaining
TB=16 k-tiles of 128 run with x in bf16 against fp8e4 stationary weights
(mixed-dtype matmul runs at full bf16 rate and is exact). Measured vs the
seeded reference: max-rel 0.0180 / rms-rel 0.0193 (gate 0.02).

All weights ship as fp8 (7.2 MB/core vs 14.7 bf16); x ships as 2 MB fp8 +
4 MB bf16. Per (n-tile, m-chunk) PSUM group: 8 DR + 16 bf16 matmuls
(~5.2us); 28 groups ~= 146us PE stream vs 191us for pure bf16.
End-to-end ~165.5us vs the 212us tuned-bf16 baseline (1.28x).

Structure: PE warmup bridges the entry preamble + first-DMA wait (first
transfers complete anywhere in 10.5-12.5us run-to-run) and must reach that
window - an idle gap before the stream drops the PE to half clock for ~3us.
Phase A covers the first 4 n-tiles as a diagonal wavefront (n-tile j starts
its k-sequence at step j) so early data demand ramps gently while x streams
in; phase B runs per n-tile with double-buffered single-DMA weight tiles
(DR slices [:, f, :, :], mixed k-tile t at [:, F + t//2, t%2, :]). Input DMAs ride the sync HWDGE ring only
(concurrent scalar-ring input DMA slows the PE stream ~19%; each dma_start
costs ~0.65us of sequencer issue, so chunks are >=256KB, ordered by PE
consumption). Output stores ride the scalar (ACT) ring. Drain = one ScalarE
activation bf16(psum*s2 + bias); within a drain, mc1's activation is
emitted first so the banks' next occupant can carry a single subsumed
start-wait. The final drain is split into 256-col chunks whose stores ride
different rings.
"""

import numpy as np
import ml_dtypes

bf16 = ml_dtypes.bfloat16
e4 = ml_dtypes.float8_e4m3  # TRN FP8_EXP4 semantics (max normal 240)

NC = 8
M, K, N = 1024, 4096, 14336
NPER = N // NC  # 1792 out-features per core
NT = NPER // 128  # 14 n-tiles
F = 8  # DoubleRow k-tiles of 256 (fp8 x); k in [0, 256*F)
KF = 256 * F
TB = (K - KF) // 128  # 16 bf16 k-tiles of 128
WS = F + TB // 2  # merged weight-tile slots per n-tile (phase B)
NA = 4  # phase-A n-tiles (4 nt x 2 m-chunks = 8 PSUM banks)
MT = 2  # m-chunks of 512

_cache = {}


def _build_nc():
    import concourse.bacc as bacc
    import concourse.mybir as mybir
    import concourse.tile as tile
    from contextlib import ExitStack

    DR = mybir.MatmulPerfMode.DoubleRow
    AF = mybir.ActivationFunctionType

    nc = bacc.Bacc("TRN2", target_bir_lowering=False, debug=False)
    x8d = nc.declare_dram_parameter("x8", [F, 128, 2, M], mybir.dt.float8e4, isOutput=False)
    xbd = nc.declare_dram_parameter("xb", [128, TB, M], mybir.dt.bfloat16, isOutput=False)
    # phase-B weights: DR slices [:, f, :, :] and mixed k-tile t at
    # [:, F + t//2, t%2, :], one tile + one DMA per n-tile
    wd = nc.declare_dram_parameter("w", [NT, 128, WS, 2, 128], mybir.dt.float8e4, isOutput=False)
    # phase-A weights repacked partition-major so one DMA spans several n-tiles
    wa8d = nc.declare_dram_parameter("wa8", [128, NA, F, 2, 128], mybir.dt.float8e4, isOutput=False)
    wamd = nc.declare_dram_parameter("wam", [128, NA, TB, 128], mybir.dt.float8e4, isOutput=False)
    bgd = nc.declare_dram_parameter("bias", [128, NT], mybir.dt.float32, isOutput=False)
    scd = nc.declare_dram_parameter("sc", [128, 1], mybir.dt.float32, isOutput=False)
    yT = nc.declare_dram_parameter("yT", [NPER, M], mybir.dt.bfloat16, isOutput=True)

    with tile.TileContext(nc) as tc, ExitStack() as ctx:
        xpool = ctx.enter_context(tc.tile_pool(name="x", bufs=1))
        wapool = ctx.enter_context(tc.tile_pool(name="wa", bufs=1))
        wpool = ctx.enter_context(tc.tile_pool(name="w", bufs=3))
        bpool = ctx.enter_context(tc.tile_pool(name="b", bufs=1))
        opool = ctx.enter_context(tc.tile_pool(name="o", bufs=4))
        pspool = ctx.enter_context(tc.tile_pool(name="ps", bufs=8, space="PSUM"))

        # PE warmup: dep-free dummy matmuls run during the entry preamble +
        # first-DMA wait (~10us before real operands land) and burn the
        # cold-clock HAM window on garbage instead of real work.
        scratch = nc.alloc_sbuf_tensor("warm_src", [128, 128], mybir.dt.bfloat16)
        ps_warm = pspool.tile([128, 128], mybir.dt.float32, tag="ps")
        for _ in range(42):
            nc.tensor.matmul(ps_warm[:, :], scratch[:, :], scratch[:, :], start=True, stop=True)

        bias_sb = bpool.tile([128, NT], mybir.dt.float32)
        sc_sb = bpool.tile([128, 1], mybir.dt.float32, tag="sc")
        nc.gpsimd.dma_start(bias_sb[:], bgd[:])
        nc.gpsimd.dma_start(sc_sb[:], scd[:])

        x8_sb = xpool.tile([128, F, 2, M], mybir.dt.float8e4, tag="x8")
        xb_sb = xpool.tile([128, TB, M], mybir.dt.bfloat16, tag="xb")
        wa8 = wapool.tile([128, NA, F, 2, 128], mybir.dt.float8e4, tag="wa8")
        wam = wapool.tile([128, NA, TB, 128], mybir.dt.float8e4, tag="wam")

        # ---- input DMA issue order (sync ring), matched to PE consumption.
        # Each dma_start occupies the sequencer ~0.65us, so >=256KB chunks;
        # concurrent input DMA on the scalar ring slows the PE stream ~19%
        # (SBUF port contention) - keep all inputs on the sync ring.
        nc.sync.dma_start(wa8[:, 0:1], wa8d[:, 0:1])
        nc.sync.dma_start(x8_sb[:, 0], x8d[0])
        nc.sync.dma_start(wa8[:, 1:2], wa8d[:, 1:2])
        nc.sync.dma_start(wa8[:, 2:4], wa8d[:, 2:4])
        nc.sync.dma_start(x8_sb[:, 1], x8d[1])
        nc.sync.dma_start(x8_sb[:, 2], x8d[2])
        nc.sync.dma_start(wam[:, 0:2], wamd[:, 0:2])
        nc.sync.dma_start(x8_sb[:, 3], x8d[3])
        nc.sync.dma_start(wam[:, 2:4], wamd[:, 2:4])
        for f in range(4, F):
            nc.sync.dma_start(x8_sb[:, f], x8d[f])
        XBC = 3  # xb DMA chunk (k-tiles)
        for c0 in range(0, TB, XBC):
            c1 = min(c0 + XBC, TB)
            nc.sync.dma_start(xb_sb[:, c0:c1, :], xbd[:, c0:c1, :])

        def mm_dr(ps, w3, f, mc, start):
            nc.tensor.matmul(
                ps[:, :],
                w3,
                x8_sb[:, f, :, mc * 512:(mc + 1) * 512],
                start=start, stop=False, perf_mode=DR,
            )

        def mm_bf(ps, w2, t, mc, stop):
            nc.tensor.matmul(
                ps[:, :],
                w2,
                xb_sb[:, t, mc * 512:(mc + 1) * 512],
                start=False, stop=stop,
            )

        def drain2(psums, nt):
            # both m-chunks of one n-tile -> one SBUF tile -> one store on
            # the scalar (ACT) HWDGE ring; bf16(psum * 2*w_scale + bias).
            # mc1's activation goes FIRST: the next occupant of these banks
            # emits its mc0 start-matmul first with the higher drain-sem
            # value, so its mc1 start-wait is subsumed and elided.
            o = opool.tile([128, M], mybir.dt.bfloat16, tag="o", name=f"o{nt}")
            for mc in (1, 0):
                nc.scalar.activation(
                    o[:, mc * 512:(mc + 1) * 512], psums[mc][:, :],
                    AF.Identity, bias=bias_sb[:, nt:nt + 1], scale=sc_sb[:, 0:1],
                )
            nc.scalar.dma_start(yT[nt * 128:(nt + 1) * 128, :], o[:])

        def drain1(ps, nt, mc):
            o = opool.tile([128, 512], mybir.dt.bfloat16, tag="oh", name=f"o{nt}_{mc}")
            nc.scalar.activation(
                o[:], ps[:, :],
                AF.Identity, bias=bias_sb[:, nt:nt + 1], scale=sc_sb[:, 0:1],
            )
            nc.scalar.dma_start(yT[nt * 128:(nt + 1) * 128, mc * 512:(mc + 1) * 512], o[:])

        # ---- Phase A: nt 0..NA-1 as a diagonal wavefront: n-tile j starts
        # its 24-item k-sequence at step j, so the PE's early data demand
        # ramps gently (1 tile + 1 x chunk before the first matmul, one new
        # item per step) instead of needing all 4 weight tiles at once.
        psA = {
            (j, mc): pspool.tile([128, 512], mybir.dt.float32, tag="ps", name=f"psA{j}_{mc}")
            for j in range(NA) for mc in range(MT)
        }
        SEQ = [("dr", f) for f in range(F)] + [("bf", t) for t in range(TB)]
        for s in range(len(SEQ) + NA - 1):
            for j in range(NA):
                i = s - j
                if not (0 <= i < len(SEQ)):
                    continue
                kind, k = SEQ[i]
                for mc in range(MT):
                    if kind == "dr":
                        mm_dr(psA[j, mc], wa8[:, j, k, :, :], k, mc, start=(k == 0))
                    else:
                        mm_bf(psA[j, mc], wam[:, j, k, :], k, mc, stop=(k == TB - 1))
                if i == len(SEQ) - 1:
                    drain2([psA[j, 0], psA[j, 1]], j)

        # ---- Phase B: per n-tile, weights double-buffered
        for nt in range(NA, NT):
            wt = wpool.tile([128, WS, 2, 128], mybir.dt.float8e4, tag="w", name=f"w{nt}")
            nc.sync.dma_start(wt[:], wd[nt])
            last = nt == NT - 1
            psb = [
                pspool.tile([128, 512], mybir.dt.float32, tag="ps", name=f"ps{nt}_{mc}")
                for mc in range(MT)
            ]
            # both banks' start-matmuls adjacent (mc1 start-wait subsumed)
            # AND both stop-matmuls adjacent at the n-tile end: the first
            # stop's sem-inc dispatch bubble lands on the second stop /
            # next start boundary instead of on a streaming matmul
            for mc in range(MT):
                mm_dr(psb[mc], wt[:, 0, :, :], 0, mc, start=True)
            for mc in range(MT):
                for f in range(1, F):
                    mm_dr(psb[mc], wt[:, f, :, :], f, mc, start=False)
                for t in range(TB - 1):
                    mm_bf(psb[mc], wt[:, F + t // 2, t % 2, :], t, mc, stop=False)
            for mc in range(MT):
                mm_bf(psb[mc], wt[:, F + (TB - 1) // 2, (TB - 1) % 2, :], TB - 1, mc, stop=True)
            for mc in range(MT):
                if last and mc == 0:
                    drain1(psb[mc], nt, mc)
                elif last:
                    # final drain split into 256-col chunks; the two stores
                    # ride different HWDGE rings so they overlap
                    oA = opool.tile([128, 256], mybir.dt.bfloat16, tag="ohA", name="ohA")
                    oB = opool.tile([128, 256], mybir.dt.bfloat16, tag="ohB", name="ohB")
                    nc.scalar.activation(
                        oA[:], psb[mc][:, 0:256],
                        AF.Identity, bias=bias_sb[:, nt:nt + 1], scale=sc_sb[:, 0:1],
                    )
                    nc.sync.dma_start(yT[nt * 128:(nt + 1) * 128, 512:768], oA[:])
                    nc.scalar.activation(
                        oB[:], psb[mc][:, 256:512],
                        AF.Identity, bias=bias_sb[:, nt:nt + 1], scale=sc_sb[:, 0:1],
                    )
                    nc.scalar.dma_start(yT[nt * 128:(nt + 1) * 128, 768:M], oB[:])
            if not last:
                drain2(psb, nt)
    nc.compile()
    return nc


def _prep_inputs(x, weight_f8, w_scale, bias):
    x2 = np.asarray(x)
    if x2.dtype != bf16:
        x2 = x2.astype(bf16)
    xm = x2.reshape(M, K)
    xT = np.ascontiguousarray(xm.T)  # [K, M] bf16

    # x fp8 part: [f, p, j, m] with k = f*256 + j*128 + p
    x8_dev = np.ascontiguousarray(
        xT[:KF].reshape(F, 2, 128, M).transpose(0, 2, 1, 3)
    ).astype(e4)  # [F, 128, 2, M]
    # x bf16 part: [p, t, m] with k = KF + t*128 + p
    xb_dev = np.ascontiguousarray(
        xT[KF:].reshape(TB, 128, M).transpose(1, 0, 2)
    )  # [128, TB, M]

    # weights: w_f8/2 is exactly fp8e4-representable (<=224); compensated by
    # scale 2*w_scale at drain. (Sub-subnormal tail rounds with max abs err
    # 2^-10 in w_f8 units - negligible.)
    wq = np.asarray(weight_f8, dtype=np.float32)
    w_half = (wq * np.float32(0.5)).astype(e4)  # [N, K] fp8

    s2 = np.float32(2.0) * np.float32(np.asarray(w_scale).astype(bf16))
    sc_dev = np.full((128, 1), s2, np.float32)

    bias_r = np.asarray(bias, dtype=np.float32).astype(bf16).astype(np.float32)

    in_maps = []
    for c in range(NC):
        part = w_half[c * NPER:(c + 1) * NPER]  # [1792, 4096] fp8
        w8_dev = part[:, :KF].reshape(NT, 128, F, 2, 128).transpose(0, 4, 2, 3, 1)
        # mixed part: slot F + t//2, sub-index j = t%2 holds k-tile t
        wm2_dev = (
            part[:, KF:].reshape(NT, 128, TB // 2, 2, 128).transpose(0, 4, 2, 3, 1)
        )
        w_dev = np.ascontiguousarray(np.concatenate([w8_dev, wm2_dev], axis=2))
        wm_dev = np.ascontiguousarray(
            part[:, KF:].reshape(NT, 128, TB, 128).transpose(0, 3, 2, 1)
        )  # [NT, p, TB, n2] (phase-A layout)
        bias_grid = np.ascontiguousarray(
            bias_r[c * NPER:(c + 1) * NPER].reshape(NT, 128).T
        )  # [128, NT]
        wa8_dev = np.ascontiguousarray(w_dev[:NA, :, :F].transpose(1, 0, 2, 3, 4))
        wam_dev = np.ascontiguousarray(wm_dev[:NA].transpose(1, 0, 2, 3))
        in_maps.append({
            "x8": x8_dev, "xb": xb_dev, "w": w_dev,
            "wa8": wa8_dev, "wam": wam_dev,
            "bias": bias_grid, "sc": sc_dev,
        })
    return in_maps


def run(x, weight_f8, w_scale, bias, trace=False, tmpdir=None):
    from concourse.bass_utils import run_bass_kernel_spmd

    if "nc" not in _cache:
        _cache["nc"] = _build_nc()
    nc = _cache["nc"]
    in_maps = _prep_inputs(x, weight_f8, w_scale, bias)
    res = run_bass_kernel_spmd(
        nc, in_maps, list(range(NC)), trace=trace, tmpdir=tmpdir
    )
    parts = [np.asarray(res.results[c]["yT"]) for c in range(NC)]  # each [1792, 1024]
    y = np.ascontiguousarray(np.concatenate(parts, axis=0).T)  # [1024, 14336]
    return y.reshape(2, 512, N), res


def kernel(x, weight_f8, w_scale, bias):
    y, _ = run(x, weight_f8, w_scale, bias)
    return y


# revision 48
# speedup vs baseline: 1.0260x; 1.0260x over previous
"""F8Linear as a column-parallel hybrid fp8/bf16 GEMM across 8 NeuronCores.

y = x @ (w_f8 * w_scale).T + bias
  x: [2, 512, 4096] bf16, w_f8: [14336, 4096] f32 (fp8-representable values),
  w_scale: scalar f32, bias: [14336] f32 -> y: [2, 512, 14336] bf16

Sharding: column-parallel - each core owns 1792 out-features; x replicated;
host gathers the 8 output slices. No collectives.

Speed trick: the TRN2 PE runs fp8e4 matmuls in DoubleRow mode at 2x the bf16
k-throughput (measured 220ns per FD=512 matmul contracting 256 k, vs 215ns
per 128 k in bf16). The weights are exactly fp8-representable: w_f8/2 lands
in TRN fp8e4 range (|w|<=224<240), so the weight side is EXACT in fp8 and
one fp32 scale (2*w_scale) is applied at PSUM drain. Only x must be
quantized to e4m3 for the DoubleRow moving operand, which costs ~2.7% rms
error if applied to all of K - too much for the 2e-2 gate. So K is split:
the first F=8 k-tiles of 256 (2048 k) run as fp8 DoubleRow; the remaining
TB=16 k-tiles of 128 run with x in bf16 against fp8e4 stationary weights
(mixed-dtype matmul runs at full bf16 rate and is exact). Measured vs the
seeded reference: max-rel 0.0180 / rms-rel 0.0193 (gate 0.02).

All weights ship as fp8 (7.2 MB/core vs 14.7 bf16); x ships as 2 MB fp8 +
4 MB bf16. Per (n-tile, m-chunk) PSUM group: 8 DR + 16 bf16 matmuls
(~5.2us); 28 groups ~= 146us PE stream vs 191us for pure bf16.
End-to-end ~165.5us vs the 212us tuned-bf16 baseline (1.28x).

Structure: PE warmup bridges the entry preamble + first-DMA wait (first
transfers complete anywhere in 10.5-12.5us run-to-run) and must reach that
window - an idle gap before the stream drops the PE to half clock for ~3us.
Phase A covers the first 4 n-tiles as a diagonal wavefront (n-tile j starts
its k-sequence at step j) so early data demand ramps gently while x streams
in; phase B runs per n-tile with double-buffered single-DMA weight tiles
(DR slices [:, f, :, :], mixed k-tile t at [:, F + t//2, t%2, :]). Input DMAs ride the sync HWDGE ring only
(concurrent scalar-ring input DMA slows the PE stream ~19%; each dma_start
costs ~0.65us of sequencer issue, so chunks are >=256KB, ordered by PE
consumption). Output stores ride the scalar (ACT) ring. Drain = one ScalarE
activation bf16(psum*s2 + bias); within a drain, mc1's activation is
emitted first so the banks' next occupant can carry a single subsumed
start-wait. The final drain is split into 256-col chunks whose stores ride
different rings.
"""

import numpy as np
import ml_dtypes

bf16 = ml_dtypes.bfloat16
e4 = ml_dtypes.float8_e4m3  # TRN FP8_EXP4 semantics (max normal 240)

NC = 8
M, K, N = 1024, 4096, 14336
NPER = N // NC  # 1792 out-features per core
NT = NPER // 128  # 14 n-tiles
F = 8  # DoubleRow k-tiles of 256 (fp8 x); k in [0, 256*F)
KF = 256 * F
TB = (K - KF) // 128  # 16 bf16 k-tiles of 128
WS = F + TB // 2  # merged weight-tile slots per n-tile (phase B)
NA = 4  # phase-A n-tiles (4 nt x 2 m-chunks = 8 PSUM banks)
MT = 2  # m-chunks of 512

_cache = {}


def _build_nc():
    import concourse.bacc as bacc
    import concourse.mybir as mybir
    import concourse.tile as tile
    from contextlib import ExitStack

    DR = mybir.MatmulPerfMode.DoubleRow
    AF = mybir.ActivationFunctionType

    nc = bacc.Bacc("TRN2", target_bir_lowering=False, debug=False)
    x8d = nc.declare_dram_parameter("x8", [F, 128, 2, M], mybir.dt.float8e4, isOutput=False)
    xbd = nc.declare_dram_parameter("xb", [128, TB, M], mybir.dt.bfloat16, isOutput=False)
    # phase-B weights: DR slices [:, f, :, :] and mixed k-tile t at
    # [:, F + t//2, t%2, :], one tile + one DMA per n-tile
    wd = nc.declare_dram_parameter("w", [NT, 128, WS, 2, 128], mybir.dt.float8e4, isOutput=False)
    # phase-A weights repacked partition-major so one DMA spans several n-tiles
    wa8d = nc.declare_dram_parameter("wa8", [128, NA, F, 2, 128], mybir.dt.float8e4, isOutput=False)
    wamd = nc.declare_dram_parameter("wam", [128, NA, TB, 128], mybir.dt.float8e4, isOutput=False)
    bgd = nc.declare_dram_parameter("bias", [128, NT], mybir.dt.float32, isOutput=False)
    scd = nc.declare_dram_parameter("sc", [128, 1], mybir.dt.float32, isOutput=False)
    yT = nc.declare_dram_parameter("yT", [NPER, M], mybir.dt.bfloat16, isOutput=True)

    with tile.TileContext(nc) as tc, ExitStack() as ctx:
        xpool = ctx.enter_context(tc.tile_pool(name="x", bufs=1))
        wapool = ctx.enter_context(tc.tile_pool(name="wa", bufs=1))
        wpool = ctx.enter_context(tc.tile_pool(name="w", bufs=3))
        bpool = ctx.enter_context(tc.tile_pool(name="b", bufs=1))
        opool = ctx.enter_context(tc.tile_pool(name="o", bufs=4))
        pspool = ctx.enter_context(tc.tile_pool(name="ps", bufs=8, space="PSUM"))

        # PE warmup: dep-free dummy matmuls run during the entry preamble +
        # first-DMA wait (~10us before real operands land) and burn the
        # cold-clock HAM window on garbage instead of real work.
        scratch = nc.alloc_sbuf_tensor("warm_src", [128, 128], mybir.dt.bfloat16)
        ps_warm = pspool.tile([128, 128], mybir.dt.float32, tag="ps")
        for _ in range(42):
            nc.tensor.matmul(ps_warm[:, :], scratch[:, :], scratch[:, :], start=True, stop=True)

        bias_sb = bpool.tile([128, NT], mybir.dt.float32)
        sc_sb = bpool.tile([128, 1], mybir.dt.float32, tag="sc")
        nc.gpsimd.dma_start(bias_sb[:], bgd[:])
        nc.gpsimd.dma_start(sc_sb[:], scd[:])

        x8_sb = xpool.tile([128, F, 2, M], mybir.dt.float8e4, tag="x8")
        xb_sb = xpool.tile([128, TB, M], mybir.dt.bfloat16, tag="xb")
        wa8 = wapool.tile([128, NA, F, 2, 128], mybir.dt.float8e4, tag="wa8")
        wam = wapool.tile([128, NA, TB, 128], mybir.dt.float8e4, tag="wam")

        # ---- input DMA issue order (sync ring), matched to PE consumption.
        # Each dma_start occupies the sequencer ~0.65us, so >=256KB chunks;
        # concurrent input DMA on the scalar ring slows the PE stream ~19%
        # (SBUF port contention) - keep all inputs on the sync ring.
        nc.sync.dma_start(wa8[:, 0:1], wa8d[:, 0:1])
        nc.sync.dma_start(x8_sb[:, 0], x8d[0])
        nc.sync.dma_start(wa8[:, 1:2], wa8d[:, 1:2])
        nc.sync.dma_start(wa8[:, 2:4], wa8d[:, 2:4])
        nc.sync.dma_start(x8_sb[:, 1], x8d[1])
        nc.sync.dma_start(x8_sb[:, 2], x8d[2])
        nc.sync.dma_start(wam[:, 0:2], wamd[:, 0:2])
        nc.sync.dma_start(x8_sb[:, 3], x8d[3])
        nc.sync.dma_start(wam[:, 2:4], wamd[:, 2:4])
        for f in range(4, F):
            nc.sync.dma_start(x8_sb[:, f], x8d[f])
        XBC = 3  # xb DMA chunk (k-tiles)
        for c0 in range(0, TB, XBC):
            c1 = min(c0 + XBC, TB)
            nc.sync.dma_start(xb_sb[:, c0:c1, :], xbd[:, c0:c1, :])

        def mm_dr(ps, w3, f, mc, start):
            nc.tensor.matmul(
                ps[:, :],
                w3,
                x8_sb[:, f, :, mc * 512:(mc + 1) * 512],
                start=start, stop=False, perf_mode=DR,
            )

        def mm_bf(ps, w2, t, mc, stop):
            nc.tensor.matmul(
                ps[:, :],
                w2,
                xb_sb[:, t, mc * 512:(mc + 1) * 512],
                start=False, stop=stop,
            )

        def drain2(psums, nt):
            # both m-chunks of one n-tile -> one SBUF tile -> one store on
            # the scalar (ACT) HWDGE ring; bf16(psum * 2*w_scale + bias).
            # mc1's activation goes FIRST: the next occupant of these banks
            # emits its mc0 start-matmul first with the higher drain-sem
            # value, so its mc1 start-wait is subsumed and elided.
            o = opool.tile([128, M], mybir.dt.bfloat16, tag="o", name=f"o{nt}")
            for mc in (1, 0):
                nc.scalar.activation(
                    o[:, mc * 512:(mc + 1) * 512], psums[mc][:, :],
                    AF.Identity, bias=bias_sb[:, nt:nt + 1], scale=sc_sb[:, 0:1],
                )
            nc.scalar.dma_start(yT[nt * 128:(nt + 1) * 128, :], o[:])

        def drain1(ps, nt, mc):
            o = opool.tile([128, 512], mybir.dt.bfloat16, tag="oh", name=f"o{nt}_{mc}")
            nc.scalar.activation(
                o[:], ps[:, :],
                AF.Identity, bias=bias_sb[:, nt:nt + 1], scale=sc_sb[:, 0:1],
            )
            nc.scalar.dma_start(yT[nt * 128:(nt + 1) * 128, mc * 512:(mc + 1) * 512], o[:])

        # ---- Phase A: nt 0..NA-1 as a diagonal wavefront: n-tile j starts
        # its 24-item k-sequence at step j, so the PE's early data demand
        # ramps gently (1 tile + 1 x chunk before the first matmul, one new
        # item per step) instead of needing all 4 weight tiles at once.
        psA = {
            (j, mc): pspool.tile([128, 512], mybir.dt.float32, tag="ps", name=f"psA{j}_{mc}")
            for j in range(NA) for mc in range(MT)
        }
        SEQ = [("dr", f) for f in range(F)] + [("bf", t) for t in range(TB)]
        for s in range(len(SEQ) + NA - 1):
            for j in range(NA):
                i = s - j
                if not (0 <= i < len(SEQ)):
                    continue
                kind, k = SEQ[i]
                for mc in range(MT):
                    if kind == "dr":
                        mm_dr(psA[j, mc], wa8[:, j, k, :, :], k, mc, start=(k == 0))
                    else:
                        mm_bf(psA[j, mc], wam[:, j, k, :], k, mc, stop=(k == TB - 1))
                if i == len(SEQ) - 1:
                    drain2([psA[j, 0], psA[j, 1]], j)

        # ---- Phase B: per n-tile, weights double-buffered
        for nt in range(NA, NT):
            wt = wpool.tile([128, WS, 2, 128], mybir.dt.float8e4, tag="w", name=f"w{nt}")
            nc.sync.dma_start(wt[:], wd[nt])
            last = nt == NT - 1
            psb = [
                pspool.tile([128, 512], mybir.dt.float32, tag="ps", name=f"ps{nt}_{mc}")
                for mc in range(MT)
            ]
            # both banks' start-matmuls adjacent: the mc0 start carries the
            # higher drain-sem wait; the mc1 start-wait is subsumed/elided
            for mc in range(MT):
                mm_dr(psb[mc], wt[:, 0, :, :], 0, mc, start=True)
            for mc in range(MT):
                for f in range(1, F):
                    mm_dr(psb[mc], wt[:, f, :, :], f, mc, start=False)
                for t in range(TB):
                    mm_bf(psb[mc], wt[:, F + t // 2, t % 2, :], t, mc, stop=(t == TB - 1))
                if last and mc == 0:
                    drain1(psb[mc], nt, mc)
                elif last:
                    # final drain split into 256-col chunks; the two stores
                    # ride different HWDGE rings so they overlap
                    oA = opool.tile([128, 256], mybir.dt.bfloat16, tag="ohA", name="ohA")
                    oB = opool.tile([128, 256], mybir.dt.bfloat16, tag="ohB", name="ohB")
                    nc.scalar.activation(
                        oA[:], psb[mc][:, 0:256],
                        AF.Identity, bias=bias_sb[:, nt:nt + 1], scale=sc_sb[:, 0:1],
                    )
                    nc.sync.dma_start(yT[nt * 128:(nt + 1) * 128, 512:768], oA[:])
                    nc.scalar.activation(
                        oB[:], psb[mc][:, 256:512],
                        AF.Identity, bias=bias_sb[:, nt:nt + 1], scale=sc_sb[:, 0:1],
                    )
                    nc.scalar.dma_start(yT[nt * 128:(nt + 1) * 128, 768:M], oB[:])
            if not last:
                drain2(psb, nt)
    nc.compile()
    return nc


def _prep_inputs(x, weight_f8, w_scale, bias):
    x2 = np.asarray(x)
    if x2.dtype != bf16:
        x2 = x2.astype(bf16)
    xm = x2.reshape(M, K)
    xT = np.ascontiguousarray(xm.T)  # [K, M] bf16

    # x fp8 part: [f, p, j, m] with k = f*256 + j*128 + p
    x8_dev = np.ascontiguousarray(
        xT[:KF].reshape(F, 2, 128, M).transpose(0, 2, 1, 3)
    ).astype(e4)  # [F, 128, 2, M]
    # x bf16 part: [p, t, m] with k = KF + t*128 + p
    xb_dev = np.ascontiguousarray(
        xT[KF:].reshape(TB, 128, M).transpose(1, 0, 2)
    )  # [128, TB, M]

    # weights: w_f8/2 is exactly fp8e4-representable (<=224); compensated by
    # scale 2*w_scale at drain. (Sub-subnormal tail rounds with max abs err
    # 2^-10 in w_f8 units - negligible.)
    wq = np.asarray(weight_f8, dtype=np.float32)
    w_half = (wq * np.float32(0.5)).astype(e4)  # [N, K] fp8

    s2 = np.float32(2.0) * np.float32(np.asarray(w_scale).astype(bf16))
    sc_dev = np.full((128, 1), s2, np.float32)

    bias_r = np.asarray(bias, dtype=np.float32).astype(bf16).astype(np.float32)

    in_maps = []
    for c in range(NC):
        part = w_half[c * NPER:(c + 1) * NPER]  # [1792, 4096] fp8
        w8_dev = part[:, :KF].reshape(NT, 128, F, 2, 128).transpose(0, 4, 2, 3, 1)
        # mixed part: slot F + t//2, sub-index j = t%2 holds k-tile t
        wm2_dev = (
            part[:, KF:].reshape(NT, 128, TB // 2, 2, 128).transpose(0, 4, 2, 3, 1)
        )
        w_dev = np.ascontiguousarray(np.concatenate([w8_dev, wm2_dev], axis=2))
        wm_dev = np.ascontiguousarray(
            part[:, KF:].reshape(NT, 128, TB, 128).transpose(0, 3, 2, 1)
        )  # [NT, p, TB, n2] (phase-A layout)
        bias_grid = np.ascontiguousarray(
            bias_r[c * NPER:(c + 1) * NPER].reshape(NT, 128).T
        )  # [128, NT]
        wa8_dev = np.ascontiguousarray(w_dev[:NA, :, :F].transpose(1, 0, 2, 3, 4))
        wam_dev = np.ascontiguousarray(wm_dev[:NA].transpose(1, 0, 2, 3))
        in_maps.append({
            "x8": x8_dev, "xb": xb_dev, "w": w_dev,
            "wa8": wa8_dev, "wam": wam_dev,
            "bias": bias_grid, "sc": sc_dev,
        })
    return in_maps


def run(x, weight_f8, w_scale, bias, trace=False, tmpdir=None):
    from concourse.bass_utils import run_bass_kernel_spmd

    if "nc" not in _cache:
        _cache["nc"] = _build_nc()
    nc = _cache["nc"]
    in_maps = _prep_inputs(x, weight_f8, w_scale, bias)
    res = run_bass_kernel_spmd(
        nc, in_maps, list(range(NC)), trace=trace, tmpdir=tmpdir
    )
    parts = [np.asarray(res.results[c]["yT"]) for c in range(NC)]  # each [1792, 1024]
    y = np.ascontiguousarray(np.concatenate(parts, axis=0).T)  # [1024, 14336]
    return y.reshape(2, 512, N), res


def kernel(x, weight_f8, w_scale, bias):
    y, _ = run(x, weight_f8, w_scale, bias)
    return y
